# revision 1
# baseline (speedup 1.0000x reference)
"""Trainium2 Bass kernel for the nn_LSTMCell problem.

Strategy: data-parallel over the batch dim (4096 -> 8 cores x 512), weights
replicated. All on-chip compute happens in "transposed" orientation
(hidden on PSUM partitions, batch on the free dim) so every matmul operand
can be DMA'd in its natural, contiguous layout:

    gate.T[h, b] = sum_k W.T[k, h] * act.T[k, b]
    matmul(out[M=h128, N=b512], lhsT=WT_tile[K=k128, M=h128], rhs=actT[K=k128, N=b512])

The host pre-packs (transposes + casts to bf16) activations and weights;
only device execution is the measured kernel. Matmuls run in bf16 with fp32
PSUM accumulation; all elementwise math and outputs are fp32.

Per core:
  phase 1: for each of 16 h-tiles: i/f/g gate matmuls (112 MMs), sigmoid/tanh,
           c1 = f*c0 + i*tanh(g)  -> c1 (fp32, kept in SBUF + DMA'd out),
           c1 cast to bf16 (matmul operand for the o gate).
  phase 2: for each of 16 h-tiles: o gate matmuls (48 MMs, incl. W_co @ c1.T),
           o = sigmoid(...), h1 = o * tanh(c1), DMA out.
"""

import numpy as np
import ml_dtypes
from contextlib import ExitStack

BF = ml_dtypes.bfloat16

N_CORES = 8
P = 128          # partition dim / k-tile size / m-tile size
BATCH = 4096
IN_DIM = 2048
HID = 2048
B = BATCH // N_CORES          # 512, batch per core = matmul free dim
KI = IN_DIM // P              # 16, k-tiles for x contraction
KH = HID // P                 # 16, k-tiles for h/c contraction
MT = HID // P                 # 16, output h-tiles

W_NAMES = ["ii", "hi", "if_", "hf", "cf", "ic", "hc", "io", "ho", "co"]


def _build(p, ki, kh, mt, b):
    import concourse.tile as tile
    from concourse import bacc, mybir

    bf16, f32 = mybir.dt.bfloat16, mybir.dt.float32
    Sig = mybir.ActivationFunctionType.Sigmoid
    Tanh = mybir.ActivationFunctionType.Tanh
    Mult = mybir.AluOpType.mult

    nc = bacc.Bacc(
        "TRN2",
        target_bir_lowering=False,
        debug=False,
        num_devices=N_CORES,
    )

    xT = nc.dram_tensor("xT", [p, ki, b], bf16, kind="ExternalInput").ap()
    hT = nc.dram_tensor("hT", [p, kh, b], bf16, kind="ExternalInput").ap()
    cT = nc.dram_tensor("cT", [p, kh, b], bf16, kind="ExternalInput").ap()
    c0T = nc.dram_tensor("c0T", [p, mt, b], f32, kind="ExternalInput").ap()
    bias = nc.dram_tensor("bias", [p, mt, 4], f32, kind="ExternalInput").ap()
    w = {
        n: nc.dram_tensor(
            f"w_{n}", [mt, p, (ki if n in ("ii", "if_", "ic", "io") else kh), p],
            bf16, kind="ExternalInput",
        ).ap()
        for n in W_NAMES
    }
    ogT = nc.dram_tensor("ogT", [p, mt, b], f32, kind="ExternalOutput").ap()
    h1T = nc.dram_tensor("h1T", [p, mt, b], f32, kind="ExternalOutput").ap()
    c1T = nc.dram_tensor("c1T", [p, mt, b], f32, kind="ExternalOutput").ap()

    with tile.TileContext(nc) as tc, ExitStack() as ctx:
        acts = ctx.enter_context(tc.tile_pool(name="acts", bufs=1))
        wpool = ctx.enter_context(tc.tile_pool(name="w", bufs=2))
        cpool = ctx.enter_context(tc.tile_pool(name="c0", bufs=2))
        tpool = ctx.enter_context(tc.tile_pool(name="temps", bufs=2))
        ppool = ctx.enter_context(tc.tile_pool(name="psum", bufs=8, space="PSUM"))

        # resident tensors. Activation loads go on gpsimd (a second DMA issue
        # queue) and are split into chunks so the first matmuls — which only
        # need the first x chunks plus one weight slab — start ~20us earlier
        # than one monolithic 6MB preload would allow.
        CH = 4  # k-tiles per DMA chunk
        xT_sb = acts.tile([p, ki, b], bf16, tag="xT")
        hT_sb = acts.tile([p, kh, b], bf16, tag="hT")
        cT_sb = acts.tile([p, kh, b], bf16, tag="cT")
        for src, dst, nk, eng in ((xT, xT_sb, ki, nc.gpsimd),
                                  (hT, hT_sb, kh, nc.sync),
                                  (cT, cT_sb, kh, nc.gpsimd)):
            ch = min(CH, nk)
            for c in range(0, nk, ch):
                eng.dma_start(dst[:, c:c + ch, :], src[:, c:c + ch, :])
        bias_sb = acts.tile([p, mt, 4], f32, tag="bias")
        nc.gpsimd.dma_start(bias_sb[:], bias[:])
        c1f_sb = acts.tile([p, mt, b], f32, tag="c1f")    # new cell state, fp32
        c1b_sb = acts.tile([p, mt, b], bf16, tag="c1b")   # bf16 copy for o-gate matmul

        def load_w(name, tag, m, chunks=1, eng=None):
            nk = w[name].shape[2]
            t = wpool.tile([p, nk, p], bf16, tag=tag)
            step = max(1, nk // chunks)
            for c in range(0, nk, step):
                (eng or nc.sync).dma_start(t[:, c:c + step], w[name][m, :, c:c + step])
            return t

        def accum(ps, w_t, act_sb, nk, first, last):
            for ko in range(nk):
                nc.tensor.matmul(
                    ps[:], lhsT=w_t[:, ko], rhs=act_sb[:, ko],
                    start=(first and ko == 0), stop=(last and ko == nk - 1),
                )

        # ---- phase 1: i/f/g gates + new cell state ----
        # x-term weights load (and matmul) first so the first m-tile's PE work
        # starts as soon as xT chunks land, while hT/cT still stream in.
        for m in range(mt):
            # m=0/m=1 slab issues go on the otherwise-idle scalar/vector
            # engines: the sync/gpsimd queues take ~650ns per dma_start, so
            # serializing ~30 early descriptors on two engines would delay the
            # DMA ramp by ~10us. Scalar/vector do no work before ~37us.
            first = 4 if m == 0 else 1
            rest = 2 if m < 2 else 1
            eng = nc.scalar if m == 0 else None
            w_ii = load_w("ii", "w0", m, chunks=first, eng=eng)
            w_if = load_w("if_", "w2", m, chunks=first, eng=eng)
            w_ic = load_w("ic", "w5", m, chunks=first, eng=eng)
            w_hi = load_w("hi", "w1", m, chunks=rest, eng=eng)
            w_hf = load_w("hf", "w3", m, chunks=rest, eng=eng)
            w_hc = load_w("hc", "w6", m, chunks=rest, eng=eng)
            w_cf = load_w("cf", "w4", m, chunks=rest, eng=eng)

            ps_i = ppool.tile([p, b], f32, tag="ps")
            ps_f = ppool.tile([p, b], f32, tag="ps")
            ps_g = ppool.tile([p, b], f32, tag="ps")
            accum(ps_i, w_ii, xT_sb, ki, True, False)
            accum(ps_f, w_if, xT_sb, ki, True, False)
            accum(ps_g, w_ic, xT_sb, ki, True, False)
            accum(ps_i, w_hi, hT_sb, kh, False, True)
            accum(ps_f, w_hf, hT_sb, kh, False, False)
            accum(ps_g, w_hc, hT_sb, kh, False, True)
            accum(ps_f, w_cf, cT_sb, kh, False, True)

            i_act = tpool.tile([p, b], f32, tag="i_act")
            nc.scalar.activation(i_act[:], ps_i[:], Sig, bias=bias_sb[:, m, 0:1])
            f_act = tpool.tile([p, b], f32, tag="f_act")
            nc.scalar.activation(f_act[:], ps_f[:], Sig, bias=bias_sb[:, m, 1:2])
            g_act = tpool.tile([p, b], f32, tag="g_act")
            nc.scalar.activation(g_act[:], ps_g[:], Tanh, bias=bias_sb[:, m, 2:3])

            c0_t = cpool.tile([p, b], f32, tag="c0")
            nc.gpsimd.dma_start(c0_t[:], c0T[:, m, :])

            t1 = tpool.tile([p, b], f32, tag="t1")
            nc.vector.tensor_mul(t1[:], f_act[:], c0_t[:])
            nc.vector.tensor_mul(i_act[:], i_act[:], g_act[:])
            c1_m = c1f_sb[:, m, :]
            nc.vector.tensor_add(c1_m, t1[:], i_act[:])
            nc.vector.tensor_copy(out=c1b_sb[:, m, :], in_=c1_m)
            nc.sync.dma_start(c1T[:, m, :], c1_m)

        # ---- phase 2: o gate + h1 ----
        for m in range(mt):
            w_io = load_w("io", "w0", m)
            w_ho = load_w("ho", "w1", m)
            w_co = load_w("co", "w2", m)

            ps_o = ppool.tile([p, b], f32, tag="ps")
            accum(ps_o, w_io, xT_sb, ki, True, False)
            accum(ps_o, w_ho, hT_sb, kh, False, False)
            accum(ps_o, w_co, c1b_sb, kh, False, True)

            o_act = tpool.tile([p, b], f32, tag="o_act")
            nc.scalar.activation(o_act[:], ps_o[:], Sig, bias=bias_sb[:, m, 3:4])
            tc1 = tpool.tile([p, b], f32, tag="tc1")
            nc.scalar.activation(tc1[:], c1f_sb[:, m, :], Tanh)
            h1_t = tpool.tile([p, b], f32, tag="h1")
            nc.vector.tensor_mul(h1_t[:], o_act[:], tc1[:])

            nc.sync.dma_start(ogT[:, m, :], o_act[:])
            nc.sync.dma_start(h1T[:, m, :], h1_t[:])

    nc.compile()
    return nc


_NC = None


def _get_nc():
    global _NC
    if _NC is None:
        _NC = _build(P, KI, KH, MT, B)
    return _NC


# ---------------- host-side packing ----------------

def _pack_actT(a, dtype):
    """(b, d) -> (128, d//128, b) with [ki, ko, b] = a[b, ko*128+ki]."""
    b, d = a.shape
    return np.ascontiguousarray(
        a.T.reshape(d // P, P, b).transpose(1, 0, 2)
    ).astype(dtype, copy=False)


def _pack_w(W):
    """(H, K) -> (H//128, 128, K//128, 128) with [mt, ki, ko, m] = W[mt*128+m, ko*128+ki]."""
    H, K = W.shape
    return np.ascontiguousarray(
        W.reshape(H // P, P, K // P, P).transpose(0, 3, 2, 1).astype(BF)
    )


def _unpack_out(o):
    """(128, mt, b) [p, m, b] -> (b, mt*128)."""
    p, m, b = o.shape
    return np.ascontiguousarray(o.transpose(2, 1, 0).reshape(b, m * p))


def kernel(x, h0, c0,
           W_ii, b_ii, W_hi, b_hi, W_if_, b_if_, W_hf, b_hf, W_cf, b_cf,
           W_ic, b_ic, W_hc, b_hc, W_io, b_io, W_ho, b_ho, W_co, b_co,
           _trace=False):
    from concourse.bass_utils import run_bass_kernel_spmd

    nc = _get_nc()

    x = np.asarray(x, dtype=np.float32)
    h0 = np.asarray(h0, dtype=np.float32)
    c0 = np.asarray(c0, dtype=np.float32)
    (W_ii, W_hi, W_if_, W_hf, W_cf, W_ic, W_hc, W_io, W_ho, W_co) = [
        np.asarray(a, dtype=np.float32)
        for a in (W_ii, W_hi, W_if_, W_hf, W_cf, W_ic, W_hc, W_io, W_ho, W_co)
    ]
    (b_ii, b_hi, b_if_, b_hf, b_cf, b_ic, b_hc, b_io, b_ho, b_co) = [
        np.asarray(a, dtype=np.float32)
        for a in (b_ii, b_hi, b_if_, b_hf, b_cf, b_ic, b_hc, b_io, b_ho, b_co)
    ]

    # combined per-gate biases, packed [p, mt, gate]
    bias = np.stack(
        [
            (b_ii + b_hi).reshape(MT, P).T,
            (b_if_ + b_hf + b_cf).reshape(MT, P).T,
            (b_ic + b_hc).reshape(MT, P).T,
            (b_io + b_ho + b_co).reshape(MT, P).T,
        ],
        axis=2,
    ).astype(np.float32)
    w_packed = {
        f"w_{n}": _pack_w(W)
        for n, W in zip(W_NAMES, [W_ii, W_hi, W_if_, W_hf, W_cf,
                                  W_ic, W_hc, W_io, W_ho, W_co])
    }

    in_maps = []
    for core in range(N_CORES):
        s = slice(core * B, (core + 1) * B)
        m = {
            "xT": _pack_actT(x[s], BF),
            "hT": _pack_actT(h0[s], BF),
            "cT": _pack_actT(c0[s], BF),
            "c0T": _pack_actT(c0[s], np.float32),
            "bias": bias,
        }
        m.update(w_packed)
        in_maps.append(m)

    res = run_bass_kernel_spmd(nc, in_maps, list(range(N_CORES)), trace=_trace)

    o_g = np.empty((BATCH, HID), np.float32)
    h1 = np.empty((BATCH, HID), np.float32)
    c1 = np.empty((BATCH, HID), np.float32)
    for core in range(N_CORES):
        s = slice(core * B, (core + 1) * B)
        o_g[s] = _unpack_out(res.results[core]["ogT"])
        h1[s] = _unpack_out(res.results[core]["h1T"])
        c1[s] = _unpack_out(res.results[core]["c1T"])
    out = (o_g, h1, c1)
    if _trace:
        return out, res
    return out



# revision 3
# speedup vs baseline: 1.5696x; 1.5696x over previous
"""Trainium2 Bass kernel for the nn_LSTMCell problem.

Strategy: data-parallel over the batch dim (4096 -> 8 cores x 512), weights
replicated. All on-chip compute happens in "transposed" orientation
(hidden on PSUM partitions, batch on the free dim) so every matmul operand
can be DMA'd in its natural, contiguous layout:

    gate.T[h, b] = sum_k W.T[k, h] * act.T[k, b]

Matmuls run in fp8e4 (DoubleRow, 2 k-tiles per instruction, ~1.8x bf16
instruction throughput) except the g-gate (tanh path: no sigmoid
attenuation, dominant error contributor), which stays bf16. All weights
(both dtypes) are pre-scaled x256 on the host so the fp8 ones sit in
e4m3's normal range (raw |W|<=0.023 would quantize as subnormals with
~20% relative error); the 1/256 is folded into the scalar-engine
activation instruction: out = func(psum/256 + bias). PSUM accumulation is
fp32; all elementwise math and outputs are fp32.

Per core:
  phase 1: per h-tile: i/f gate fp8 matmuls + g gate bf16 matmuls,
           sigmoid/tanh, c1 = f*c0 + i*tanh(g) (fp32, kept in SBUF +
           DMA'd out), c1 cast to fp8 (o-gate matmul operand).
  phase 2: per h-tile: o gate fp8 matmuls (incl. W_co @ c1.T),
           o = sigmoid(...), h1 = o * tanh(c1), DMA out.
"""

import numpy as np
import ml_dtypes
from contextlib import ExitStack

BF = ml_dtypes.bfloat16
F8 = ml_dtypes.float8_e4m3   # TRN FP8_EXP4 (max +-240)
W_SCALE = 256.0              # weights pre-scaled into e4m3 normal range

N_CORES = 8
P = 128          # partition dim / k-tile size / m-tile size
BATCH = 4096
IN_DIM = 2048
HID = 2048
B = BATCH // N_CORES          # 512, batch per core = matmul free dim
KI = IN_DIM // P              # 16, k-tiles for x contraction
KH = HID // P                 # 16, k-tiles for h/c contraction
MT = HID // P                 # 16, output h-tiles

W_NAMES = ["ii", "hi", "if_", "hf", "cf", "ic", "hc", "io", "ho", "co"]
X_NAMES = ("ii", "if_", "ic", "io")   # weights contracting over x
# matmuls run in fp8 DoubleRow except the g-gate (tanh path)
FP8_SET = frozenset(W_NAMES) - {"ic", "hc"}


def _build(p, ki, kh, mt, b, fp8_set):
    import concourse.tile as tile
    from concourse import bacc, mybir

    bf16, f32, f8 = mybir.dt.bfloat16, mybir.dt.float32, mybir.dt.float8e4
    Sig = mybir.ActivationFunctionType.Sigmoid
    Tanh = mybir.ActivationFunctionType.Tanh
    DR = mybir.MatmulPerfMode.DoubleRow
    inv_s = 1.0 / W_SCALE

    nc = bacc.Bacc(
        "TRN2",
        target_bir_lowering=False,
        debug=False,
        num_devices=N_CORES,
    )

    def wdt(n):
        return f8 if n in fp8_set else bf16

    # which activation dtypes are needed
    need8 = {a: any(n in fp8_set for n in ns) for a, ns in
             (("x", X_NAMES), ("h", ("hi", "hf", "hc", "ho")), ("c", ("cf",)))}
    needb = {a: any(n not in fp8_set for n in ns) for a, ns in
             (("x", X_NAMES), ("h", ("hi", "hf", "hc", "ho")), ("c", ("cf",)))}

    dram_act = {}
    for a in ("x", "h", "c"):
        if need8[a]:
            dram_act[a + "8"] = nc.dram_tensor(
                a + "T8", [p, ki if a == "x" else kh, b], f8, kind="ExternalInput").ap()
        if needb[a]:
            dram_act[a + "b"] = nc.dram_tensor(
                a + "Tb", [p, ki if a == "x" else kh, b], bf16, kind="ExternalInput").ap()
    c0T = nc.dram_tensor("c0T", [p, mt, b], f32, kind="ExternalInput").ap()
    bias = nc.dram_tensor("bias", [p, mt, 4], f32, kind="ExternalInput").ap()
    w = {
        n: nc.dram_tensor(
            f"w_{n}", [mt, p, (ki if n in X_NAMES else kh), p],
            wdt(n), kind="ExternalInput",
        ).ap()
        for n in W_NAMES
    }
    ogT = nc.dram_tensor("ogT", [p, mt, b], f32, kind="ExternalOutput").ap()
    h1T = nc.dram_tensor("h1T", [p, mt, b], f32, kind="ExternalOutput").ap()
    c1T = nc.dram_tensor("c1T", [p, mt, b], f32, kind="ExternalOutput").ap()

    with tile.TileContext(nc) as tc, ExitStack() as ctx:
        acts = ctx.enter_context(tc.tile_pool(name="acts", bufs=1))
        wpool = ctx.enter_context(tc.tile_pool(name="w", bufs=2))
        cpool = ctx.enter_context(tc.tile_pool(name="c0", bufs=2))
        tpool = ctx.enter_context(tc.tile_pool(name="temps", bufs=2))
        ppool = ctx.enter_context(tc.tile_pool(name="psum", bufs=8, space="PSUM"))

        # resident activations. Loads are split into chunks across two DMA
        # issue queues so the first matmuls start before everything lands.
        CH = 4  # k-tiles per DMA chunk
        act_sb = {}
        load_list = []
        for key, src in dram_act.items():
            nk = src.shape[1]
            dst = acts.tile([p, nk, b], f8 if key.endswith("8") else bf16, tag=key)
            act_sb[key] = dst
            load_list.append((src, dst, nk))
        for i, (src, dst, nk) in enumerate(load_list):
            eng = nc.gpsimd if i % 2 == 0 else nc.sync
            for c in range(0, nk, CH):
                eng.dma_start(dst[:, c:c + CH, :], src[:, c:c + CH, :])
        bias_sb = acts.tile([p, mt, 4], f32, tag="bias")
        nc.gpsimd.dma_start(bias_sb[:], bias[:])
        c1f_sb = acts.tile([p, mt, b], f32, tag="c1f")    # new cell state, fp32
        # copy of c1 in the o-gate matmul operand dtype
        c1m_dt = f8 if "co" in fp8_set else bf16
        c1m_sb = acts.tile([p, mt, b], c1m_dt, tag="c1m")

        def load_w(name, tag, m, chunks=1, eng=None):
            nk = w[name].shape[2]
            t = wpool.tile([p, nk, p], wdt(name), tag=tag)
            step = max(1, nk // chunks)
            for c in range(0, nk, step):
                (eng or nc.sync).dma_start(t[:, c:c + step], w[name][m, :, c:c + step])
            return t

        def accum(ps, name, w_t, act_key, first, last):
            fp8 = name in fp8_set
            if isinstance(act_key, str):
                a = act_sb[act_key + ("8" if fp8 else "b")]
            else:
                a = act_key
            nk = w_t.shape[1]
            if fp8:
                for t in range(0, nk, 2):
                    nc.tensor.matmul(
                        ps[:], lhsT=w_t[:, t:t + 2, :], rhs=a[:, t:t + 2, :],
                        start=(first and t == 0), stop=(last and t == nk - 2),
                        perf_mode=DR,
                    )
            else:
                for t in range(nk):
                    nc.tensor.matmul(
                        ps[:], lhsT=w_t[:, t], rhs=a[:, t],
                        start=(first and t == 0), stop=(last and t == nk - 1),
                    )

        # ---- phase 1: i/f/g gates + new cell state ----
        for m in range(mt):
            # m=0/m=1 slab issues go on the otherwise-idle scalar engine so
            # the early DMA ramp isn't serialized behind the act loads.
            first = 4 if m == 0 else 1
            rest = 2 if m < 2 else 1
            eng = nc.scalar if m == 0 else None
            w_ii = load_w("ii", "w0", m, chunks=first, eng=eng)
            w_if = load_w("if_", "w2", m, chunks=first, eng=eng)
            w_ic = load_w("ic", "w5", m, chunks=first, eng=eng)
            w_hi = load_w("hi", "w1", m, chunks=rest, eng=eng)
            w_hf = load_w("hf", "w3", m, chunks=rest, eng=eng)
            w_hc = load_w("hc", "w6", m, chunks=rest, eng=eng)
            w_cf = load_w("cf", "w4", m, chunks=rest, eng=eng)

            ps_i = ppool.tile([p, b], f32, tag="ps")
            ps_f = ppool.tile([p, b], f32, tag="ps")
            ps_g = ppool.tile([p, b], f32, tag="ps")
            accum(ps_i, "ii", w_ii, "x", True, False)
            accum(ps_f, "if_", w_if, "x", True, False)
            accum(ps_g, "ic", w_ic, "x", True, False)
            accum(ps_i, "hi", w_hi, "h", False, True)
            accum(ps_f, "hf", w_hf, "h", False, False)
            accum(ps_g, "hc", w_hc, "h", False, True)
            accum(ps_f, "cf", w_cf, "c", False, True)

            i_act = tpool.tile([p, b], f32, tag="i_act")
            nc.scalar.activation(i_act[:], ps_i[:], Sig,
                                 bias=bias_sb[:, m, 0:1], scale=inv_s)
            f_act = tpool.tile([p, b], f32, tag="f_act")
            nc.scalar.activation(f_act[:], ps_f[:], Sig,
                                 bias=bias_sb[:, m, 1:2], scale=inv_s)
            g_act = tpool.tile([p, b], f32, tag="g_act")
            nc.scalar.activation(g_act[:], ps_g[:], Tanh,
                                 bias=bias_sb[:, m, 2:3], scale=inv_s)

            c0_t = cpool.tile([p, b], f32, tag="c0")
            nc.gpsimd.dma_start(c0_t[:], c0T[:, m, :])

            t1 = tpool.tile([p, b], f32, tag="t1")
            nc.vector.tensor_mul(t1[:], f_act[:], c0_t[:])
            nc.vector.tensor_mul(i_act[:], i_act[:], g_act[:])
            c1_m = c1f_sb[:, m, :]
            nc.vector.tensor_add(c1_m, t1[:], i_act[:])
            nc.vector.tensor_copy(out=c1m_sb[:, m, :], in_=c1_m)
            nc.sync.dma_start(c1T[:, m, :], c1_m)

        # ---- phase 2: o gate + h1 ----
        for m in range(mt):
            w_io = load_w("io", "w0", m)
            w_ho = load_w("ho", "w1", m)
            w_co = load_w("co", "w2", m)

            ps_o = ppool.tile([p, b], f32, tag="ps")
            accum(ps_o, "io", w_io, "x", True, False)
            accum(ps_o, "ho", w_ho, "h", False, False)
            accum(ps_o, "co", w_co, c1m_sb, False, True)

            o_act = tpool.tile([p, b], f32, tag="o_act")
            nc.scalar.activation(o_act[:], ps_o[:], Sig,
                                 bias=bias_sb[:, m, 3:4], scale=inv_s)
            tc1 = tpool.tile([p, b], f32, tag="tc1")
            nc.scalar.activation(tc1[:], c1f_sb[:, m, :], Tanh)
            h1_t = tpool.tile([p, b], f32, tag="h1")
            nc.vector.tensor_mul(h1_t[:], o_act[:], tc1[:])

            nc.sync.dma_start(ogT[:, m, :], o_act[:])
            nc.gpsimd.dma_start(h1T[:, m, :], h1_t[:])

    nc.compile()
    return nc


_NC = None
_NC_KEY = None


def _get_nc():
    global _NC, _NC_KEY
    key = frozenset(FP8_SET)
    if _NC is None or _NC_KEY != key:
        _NC = _build(P, KI, KH, MT, B, key)
        _NC_KEY = key
    return _NC


# ---------------- host-side packing ----------------

def _pack_actT(a, dtype):
    """(b, d) -> (128, d//128, b) with [ki, ko, b] = a[b, ko*128+ki]."""
    b, d = a.shape
    return np.ascontiguousarray(
        a.T.reshape(d // P, P, b).transpose(1, 0, 2)
    ).astype(dtype, copy=False)


def _pack_w(W, dtype):
    """(H, K) -> (H//128, 128, K//128, 128) with [mt, ki, ko, m] = s*W[mt*128+m, ko*128+ki]."""
    H, K = W.shape
    return np.ascontiguousarray(
        (W.reshape(H // P, P, K // P, P) * W_SCALE)
        .transpose(0, 3, 2, 1).astype(dtype)
    )


def _unpack_out(o):
    """(128, mt, b) [p, m, b] -> (b, mt*128)."""
    p, m, b = o.shape
    return np.ascontiguousarray(o.transpose(2, 1, 0).reshape(b, m * p))


def kernel(x, h0, c0,
           W_ii, b_ii, W_hi, b_hi, W_if_, b_if_, W_hf, b_hf, W_cf, b_cf,
           W_ic, b_ic, W_hc, b_hc, W_io, b_io, W_ho, b_ho, W_co, b_co,
           _trace=False):
    from concourse.bass_utils import run_bass_kernel_spmd

    nc = _get_nc()

    x = np.asarray(x, dtype=np.float32)
    h0 = np.asarray(h0, dtype=np.float32)
    c0 = np.asarray(c0, dtype=np.float32)
    Ws = dict(zip(W_NAMES, [W_ii, W_hi, W_if_, W_hf, W_cf,
                            W_ic, W_hc, W_io, W_ho, W_co]))
    Ws = {n: np.asarray(a, dtype=np.float32) for n, a in Ws.items()}
    (b_ii, b_hi, b_if_, b_hf, b_cf, b_ic, b_hc, b_io, b_ho, b_co) = [
        np.asarray(a, dtype=np.float32)
        for a in (b_ii, b_hi, b_if_, b_hf, b_cf, b_ic, b_hc, b_io, b_ho, b_co)
    ]

    # combined per-gate biases, packed [p, mt, gate]
    bias = np.stack(
        [
            (b_ii + b_hi).reshape(MT, P).T,
            (b_if_ + b_hf + b_cf).reshape(MT, P).T,
            (b_ic + b_hc).reshape(MT, P).T,
            (b_io + b_ho + b_co).reshape(MT, P).T,
        ],
        axis=2,
    ).astype(np.float32)
    w_packed = {
        f"w_{n}": _pack_w(W, F8 if n in FP8_SET else BF)
        for n, W in Ws.items()
    }

    need8 = {"x": any(n in FP8_SET for n in X_NAMES),
             "h": any(n in FP8_SET for n in ("hi", "hf", "hc", "ho")),
             "c": "cf" in FP8_SET}
    needb = {"x": any(n not in FP8_SET for n in X_NAMES),
             "h": any(n not in FP8_SET for n in ("hi", "hf", "hc", "ho")),
             "c": "cf" not in FP8_SET}

    in_maps = []
    for core in range(N_CORES):
        s = slice(core * B, (core + 1) * B)
        m = {"c0T": _pack_actT(c0[s], np.float32), "bias": bias}
        for a, full in (("x", x), ("h", h0), ("c", c0)):
            if need8[a]:
                m[a + "T8"] = _pack_actT(full[s], F8)
            if needb[a]:
                m[a + "Tb"] = _pack_actT(full[s], BF)
        m.update(w_packed)
        in_maps.append(m)

    res = run_bass_kernel_spmd(nc, in_maps, list(range(N_CORES)), trace=_trace)

    o_g = np.empty((BATCH, HID), np.float32)
    h1 = np.empty((BATCH, HID), np.float32)
    c1 = np.empty((BATCH, HID), np.float32)
    for core in range(N_CORES):
        s = slice(core * B, (core + 1) * B)
        o_g[s] = _unpack_out(res.results[core]["ogT"])
        h1[s] = _unpack_out(res.results[core]["h1T"])
        c1[s] = _unpack_out(res.results[core]["c1T"])
    out = (o_g, h1, c1)
    if _trace:
        return out, res
    return out
